# revision 5
# baseline (speedup 1.0000x reference)
"""Trainium2 Bass kernel for DRModel: ragged basket-pool + masked GRU.

Computation (matches the jax reference):
  pooled[b,s,:] = mean over valid k of encode[items[b,s,k]]   (basket pooling)
  GRU over s with packed-sequence masking:
    h' = where(s < len[b], GRUCell(pooled[b,s], h), h)
    y[b,s] = where(s < len[b], h', 0)

Sharding: data-parallel over batch, 32 users per core, 8 cores.
Embedding table (cast to bf16) + GRU weights replicated.

Device strategy per core:
  * Two-level gather built on the Q7 dma_gather ucode (int16 indices):
    pass 1 gathers only the VALID (unmasked) tokens, compacted per
    25000-row table quarter (indices fit int16), into an internal HBM
    staging buffer [~20K, 128]; pass 2 gathers from staging (ids < 20480
    fit int16) back into the static basket-ordered token layout
    [token-partition, tile, D] that the pooling matmuls expect.
  * Pooling: PE matmul emb[tokens,D] vs host-built [tokens, 6] block
    weight matrix (mask & 1/len folded) -> psum [D, baskets], basket
    order s-major (col = s*32 + b).
  * gx = W_ih @ x precomputed in 480-col chunks (= 15 GRU steps), biases
    folded; z-gate weights/bias negated so sigmoid gives (1-z) directly.
  * GRU per step: 3 bf16 matmuls (r,z,n) vs h_bf; then
      a_rz = psum_rz + gx_rz ; sig = sigmoid(a_rz)        (DVE, ACT)
      ghn  = psum_n + b_hhn   (ACT Identity+bias, off critical path)
      n    = tanh(sig_r * ghn + gx_n)                     (DVE, DVE, ACT)
      p    = mask * sig_zc                                (GpSimd, off path)
      h'   = h + p * (n - h)                              (DVE x3)
      y_s  = mstrip_s * transpose(h')       (PE transpose + ACT scale-copy)
"""

import numpy as np

B, S, K, D, V = 256, 50, 20, 128, 100000
NCORES = 8
BL = B // NCORES          # 32 users per core
NB = BL * S               # 1600 baskets per core
BPT = 6                   # baskets per token-tile
TPT = BPT * K             # 120 tokens per tile
NT = (NB + BPT - 1) // BPT            # 267 tiles
NQ = 4                    # table quarters (int16 range)
QROWS = V // NQ           # 25000
L1 = 5120                 # staged rows per quarter (mean ~4200, sigma ~56)
NSTAGE = NQ * L1          # 20480 staging rows (< 32768 for int16 pass 2)

# psum chunking: 80 tiles = 480 cols = 15 steps per chunk (last ragged)
CH_TILES = 80
CH_COLS = CH_TILES * BPT              # 480
CH_STEPS = CH_COLS // BL              # 15
CHUNKS = []  # (tile0, ntiles, col0, ncols_pad, step0, nsteps)
t0 = 0
while t0 < NT:
    nt = min(CH_TILES, NT - t0)
    col0 = t0 * BPT
    s0 = col0 // BL
    CHUNKS.append((t0, nt, col0, nt * BPT, s0, min(S - s0, CH_STEPS)))
    t0 += nt
P2PAD = CH_TILES * 128                # idx slots per pass-2 chunk
GMAX = 1024                           # max idx per dma_gather (desc ring)

_CACHE = {}


def _build():
    if "nc" in _CACHE:
        return _CACHE["nc"]
    import concourse.bacc as bacc
    import concourse.mybir as mybir
    import concourse.tile as tile
    from concourse.masks import make_identity

    f32 = mybir.dt.float32
    bf16 = mybir.dt.bfloat16
    i16 = mybir.dt.int16
    AF = mybir.ActivationFunctionType

    nc = bacc.Bacc("TRN2", target_bir_lowering=False, debug=False,
                   num_devices=NCORES)

    table = nc.dram_tensor("table", [V, D], bf16, kind="ExternalInput")
    p1_d = nc.dram_tensor("p1idx", [NQ, 128, L1 // 16], i16, kind="ExternalInput")
    p2_d = nc.dram_tensor("p2idx", [len(CHUNKS), 128, P2PAD // 16], i16,
                          kind="ExternalInput")
    wmat_d = nc.dram_tensor("wmat", [128, NT * BPT], bf16, kind="ExternalInput")
    wihT_d = nc.dram_tensor("wihT", [3, D, D], bf16, kind="ExternalInput")
    whhT_d = nc.dram_tensor("whhT", [3, D, D], bf16, kind="ExternalInput")
    bias_d = nc.dram_tensor("bias4", [D, 4], f32, kind="ExternalInput")
    mask_d = nc.dram_tensor("mask", [D, NB], f32, kind="ExternalInput")
    mstrip_d = nc.dram_tensor("mstrip", [BL, S], f32, kind="ExternalInput")
    h0_d = nc.dram_tensor("h0T", [D, BL], f32, kind="ExternalInput")
    stag_d = nc.dram_tensor("stag", [NSTAGE, D], bf16)  # internal HBM
    y_d = nc.dram_tensor("y", [BL, S * D], f32, kind="ExternalOutput")
    hout_d = nc.dram_tensor("hout", [BL, D], f32, kind="ExternalOutput")

    with tile.TileContext(nc) as tc:
        with (
            tc.tile_pool(name="const", bufs=1) as cp,
            tc.tile_pool(name="big", bufs=1) as bigp,
            tc.tile_pool(name="stg", bufs=2) as sp,
            tc.tile_pool(name="emb", bufs=2) as ep,
            tc.tile_pool(name="gru", bufs=3) as grp,
            tc.tile_pool(name="hh", bufs=2) as hp,
            tc.tile_pool(name="ppool", bufs=2, space="PSUM") as ppp,
            tc.tile_pool(name="pgx", bufs=2, space="PSUM") as pgx,
            tc.tile_pool(name="pgh", bufs=2, space="PSUM") as pgh,
            tc.tile_pool(name="ptr", bufs=2, space="PSUM") as ptr,
        ):
            # ---- constants ----
            wih_sb = cp.tile([D, 3 * D], bf16, tag="wih")
            whh_sb = cp.tile([D, 3 * D], bf16, tag="whh")
            for g in range(3):
                nc.sync.dma_start(out=wih_sb[:, g * D:(g + 1) * D], in_=wihT_d[g])
                nc.sync.dma_start(out=whh_sb[:, g * D:(g + 1) * D], in_=whhT_d[g])
            bias_sb = cp.tile([D, 4], f32, tag="bias")
            nc.sync.dma_start(out=bias_sb[:], in_=bias_d[:])
            mask_sb = cp.tile([D, NB], f32, tag="mask")
            nc.sync.dma_start(out=mask_sb[:], in_=mask_d[:])
            mstrip_sb = cp.tile([BL, S], f32, tag="mstrip")
            nc.sync.dma_start(out=mstrip_sb[:], in_=mstrip_d[:])
            ident = cp.tile([128, 128], f32, tag="ident")
            make_identity(nc, ident[:])
            h_first = hp.tile([D, BL], f32, tag="h")
            nc.sync.dma_start(out=h_first[:], in_=h0_d[:])
            wmat_sb = cp.tile([128, NT * BPT], bf16, tag="wmat")
            nc.sync.dma_start(out=wmat_sb[:], in_=wmat_d[:])

            y_sb = bigp.tile([BL, S * D], f32, tag="y")

            # ---- pass 1: gather valid tokens per quarter into HBM staging ----
            NS1 = L1 // GMAX
            for q in range(NQ):
                i1 = sp.tile([128, NS1 * (GMAX // 16)], mybir.dt.int16, tag="i1")
                nc.sync.dma_start(out=i1[:], in_=p1_d[q])
                s1 = sp.tile([128, (L1 // 128) * D], bf16, tag="s1")
                s1v = s1[:].rearrange("p (n d) -> p n d", d=D)
                for j in range(NS1):
                    nc.gpsimd.dma_gather(
                        out_ap=s1v[:, j * (GMAX // 128):(j + 1) * (GMAX // 128)],
                        in_ap=table[q * QROWS:(q + 1) * QROWS, :],
                        idxs_ap=i1[:, j * (GMAX // 16):(j + 1) * (GMAX // 16)],
                        num_idxs=GMAX, num_idxs_reg=GMAX, elem_size=D,
                    )
                nc.sync.dma_start(
                    out=stag_d[q * L1:(q + 1) * L1].rearrange(
                        "(n p) d -> p n d", p=128),
                    in_=s1v)

            # ---- pass 2 + pooling + gx, chunk by chunk ----
            pooled_c = []
            gxrz_c = []
            gxn_c = []
            for ci, (ct0, cnt, col0, ncp, s0, nst) in enumerate(CHUNKS):
                nidx = cnt * 128
                nsub = (nidx + GMAX - 1) // GMAX
                i2 = sp.tile([128, (P2PAD // GMAX) * (GMAX // 16)],
                             mybir.dt.int16, tag="i2")
                nc.sync.dma_start(out=i2[:], in_=p2_d[ci])
                em = ep.tile([128, CH_TILES * D], bf16, tag="emb")
                emv = em[:].rearrange("p (n d) -> p n d", d=D)
                for j in range(nsub):
                    nj = min(GMAX, nidx - j * GMAX)
                    nc.gpsimd.dma_gather(
                        out_ap=emv[:, j * (GMAX // 128):
                                   j * (GMAX // 128) + nj // 128],
                        in_ap=stag_d[:],
                        idxs_ap=i2[:, j * (GMAX // 16):
                                   j * (GMAX // 16) + nj // 16],
                        num_idxs=nj, num_idxs_reg=nj, elem_size=D,
                    )
                pp = ppp.tile([D, ncp], f32, tag="pp")
                for j in range(cnt):
                    t = ct0 + j
                    nc.tensor.matmul(
                        out=pp[:, j * BPT:(j + 1) * BPT],
                        lhsT=em[:TPT, j * D:(j + 1) * D],
                        rhs=wmat_sb[:TPT, t * BPT:(t + 1) * BPT],
                        start=True, stop=True,
                    )
                pool_sb = bigp.tile([D, ncp], bf16, tag=f"pool{ct0}")
                nc.vector.tensor_copy(pool_sb[:], pp[:])
                pooled_c.append(pool_sb)

                ncols = nst * BL
                gxrz = bigp.tile([D, nst * 2 * BL], f32, tag=f"gxrz{ct0}")
                gxn = bigp.tile([D, ncols], f32, tag=f"gxn{ct0}")
                gxrz_v = gxrz[:].rearrange("p (s h b) -> p s h b", h=2, b=BL)
                for g in range(3):
                    px = pgx.tile([D, ncols], f32, tag="px")
                    nc.tensor.matmul(
                        out=px[:],
                        lhsT=wih_sb[:, g * D:(g + 1) * D],
                        rhs=pool_sb[:, :ncols],
                        start=True, stop=True,
                    )
                    if g < 2:
                        dst = gxrz_v[:, :, g, :]
                    else:
                        dst = gxn[:]
                    nc.vector.tensor_scalar_add(dst, px[:], bias_sb[:, g:g + 1])
                gxrz_c.append(gxrz)
                gxn_c.append(gxn)

            # ---- GRU ----
            h_prev = h_first
            h_bf = hp.tile([D, BL], mybir.dt.bfloat16, tag="hbf")
            nc.vector.tensor_copy(h_bf[:], h_prev[:])
            for s in range(S):
                ci = min(s // CH_STEPS, len(CHUNKS) - 1)
                sl = s - CHUNKS[ci][4]
                gxrz = gxrz_c[ci]
                gxn = gxn_c[ci]

                pg = pgh.tile([D, 3 * BL], f32, tag="pg")
                for g in range(3):
                    nc.tensor.matmul(
                        out=pg[:, g * BL:(g + 1) * BL],
                        lhsT=whh_sb[:, g * D:(g + 1) * D],
                        rhs=h_bf[:],
                        start=True, stop=True,
                    )
                a_rz = grp.tile([D, 2 * BL], f32, tag="a_rz")
                nc.vector.tensor_add(a_rz[:], pg[:, 0:2 * BL],
                                     gxrz[:, sl * 2 * BL:(sl + 1) * 2 * BL])
                sig = grp.tile([D, 2 * BL], f32, tag="sig")
                nc.scalar.activation(sig[:], a_rz[:], AF.Sigmoid)
                ghn = grp.tile([D, BL], f32, tag="ghn")
                nc.scalar.activation(ghn[:], pg[:, 2 * BL:3 * BL], AF.Identity,
                                     bias=bias_sb[:, 3:4])
                p = grp.tile([D, BL], f32, tag="p")
                nc.gpsimd.tensor_mul(p[:], sig[:, BL:2 * BL],
                                     mask_sb[:, s * BL:(s + 1) * BL])
                rn = grp.tile([D, BL], f32, tag="rn")
                nc.vector.tensor_mul(rn[:], sig[:, 0:BL], ghn[:])
                npre = grp.tile([D, BL], f32, tag="npre")
                nc.vector.tensor_add(npre[:], rn[:],
                                     gxn[:, sl * BL:(sl + 1) * BL])
                nt_ = grp.tile([D, BL], f32, tag="nt")
                nc.scalar.activation(nt_[:], npre[:], AF.Tanh)
                u = grp.tile([D, BL], f32, tag="u")
                nc.vector.tensor_sub(u[:], nt_[:], h_prev[:])
                pu = grp.tile([D, BL], f32, tag="pu")
                nc.vector.tensor_mul(pu[:], p[:], u[:])
                h_next = hp.tile([D, BL], f32, tag="h")
                nc.vector.tensor_add(h_next[:], h_prev[:], pu[:])
                h_bf = hp.tile([D, BL], mybir.dt.bfloat16, tag="hbf")
                nc.vector.tensor_copy(h_bf[:], h_next[:])

                tr = ptr.tile([BL, D], f32, tag="tr")
                nc.tensor.transpose(out=tr[:], in_=h_next[:], identity=ident[:])
                nc.scalar.activation(y_sb[:, s * D:(s + 1) * D], tr[:],
                                     AF.Identity, scale=mstrip_sb[:, s:s + 1])
                h_prev = h_next

            hout_sb = cp.tile([BL, D], f32, tag="hout")
            tr = ptr.tile([BL, D], f32, tag="tr")
            nc.tensor.transpose(out=tr[:], in_=h_prev[:], identity=ident[:])
            nc.vector.tensor_copy(hout_sb[:], tr[:])

            nc.sync.dma_start(out=y_d[:], in_=y_sb[:])
            nc.sync.dma_start(out=hout_d[:], in_=hout_sb[:])

    nc.compile()
    _CACHE["nc"] = nc
    return nc


def _wrap16(flat, ncols):
    """int16 idx list -> [128, ncols] wrapped in 16 partitions, replicated 8x."""
    out = np.zeros((16, ncols), np.int16)
    n = len(flat)
    assert n % 16 == 0 and n // 16 <= ncols
    out[:, :n // 16] = flat.reshape(n // 16, 16).T
    return np.tile(out, (8, 1))


def _host_prep(items, basket_len, lengths, encode, w_ih, w_hh, b_ih, b_hh, h0):
    """Build per-core input maps."""
    import ml_dtypes
    bf = ml_dtypes.bfloat16

    items = np.asarray(items).astype(np.int64)
    basket_len = np.asarray(basket_len).astype(np.int64)
    lengths = np.asarray(lengths).astype(np.int64)
    encode = np.asarray(encode, dtype=np.float32)
    w_ih = np.asarray(w_ih, dtype=np.float32)
    w_hh = np.asarray(w_hh, dtype=np.float32)
    b_ih = np.asarray(b_ih, dtype=np.float32)
    b_hh = np.asarray(b_hh, dtype=np.float32)
    h0 = np.asarray(h0, dtype=np.float32)

    table_bf = np.ascontiguousarray(encode.astype(bf))

    wihT = np.stack([w_ih[g * D:(g + 1) * D].T.copy() for g in range(3)])
    whhT = np.stack([w_hh[g * D:(g + 1) * D].T.copy() for g in range(3)])
    wihT[1] = -wihT[1]
    whhT[1] = -whhT[1]
    bias4 = np.zeros((D, 4), np.float32)
    bias4[:, 0] = b_ih[0:D] + b_hh[0:D]
    bias4[:, 1] = -(b_ih[D:2 * D] + b_hh[D:2 * D])
    bias4[:, 2] = b_ih[2 * D:3 * D]
    bias4[:, 3] = b_hh[2 * D:3 * D]
    wihT = np.ascontiguousarray(wihT.astype(bf))
    whhT = np.ascontiguousarray(whhT.astype(bf))

    karange = np.arange(K)[None, None, :]
    vmask_bk = karange < basket_len[..., None]              # [B,S,K] valid slots
    wgt = vmask_bk.astype(np.float32) / basket_len[..., None].astype(np.float32)

    NPOS = NT * 128
    in_maps = []
    for c in range(NCORES):
        bsl = slice(c * BL, (c + 1) * BL)
        # basket = s*BL + b; token position = tile*128 + (cb*K + k)
        it_c = np.transpose(items[bsl], (1, 0, 2)).reshape(NB, K)
        wg_c = np.transpose(wgt[bsl], (1, 0, 2)).reshape(NB, K)
        vm_c = np.transpose(vmask_bk[bsl], (1, 0, 2)).reshape(NB, K)
        nbp = NT * BPT
        it_pad = np.zeros((nbp, K), np.int64)
        wg_pad = np.zeros((nbp, K), np.float32)
        vm_pad = np.zeros((nbp, K), bool)
        it_pad[:NB], wg_pad[:NB], vm_pad[:NB] = it_c, wg_c, vm_c
        # positions: [NT, 128] with cols 0..119 = (cb, k), 120..127 pad
        rows_pos = np.zeros((NT, 128), np.int64)
        rows_pos[:, :TPT] = it_pad.reshape(NT, TPT)
        vm_pos = np.zeros((NT, 128), bool)
        vm_pos[:, :TPT] = vm_pad.reshape(NT, TPT)
        rows_flat = rows_pos.reshape(NPOS)
        vm_flat = vm_pos.reshape(NPOS)

        quarter = rows_flat // QROWS
        stagidx = np.zeros(NPOS, np.int64)
        p1idx = np.zeros((NQ, 128, L1 // 16), np.int16)
        for q in range(NQ):
            sel = np.nonzero(vm_flat & (quarter == q))[0]
            cnt = len(sel)
            assert cnt <= L1, f"quarter {q} overflow: {cnt} > {L1}"
            lst = np.zeros(L1, np.int16)
            lst[:cnt] = (rows_flat[sel] - q * QROWS).astype(np.int16)
            for j in range(L1 // GMAX):
                p1idx[q, :, j * (GMAX // 16):(j + 1) * (GMAX // 16)] = \
                    _wrap16(lst[j * GMAX:(j + 1) * GMAX], GMAX // 16)
            stagidx[sel] = q * L1 + np.arange(cnt)
        # pass 2: per chunk, idx per position (tile-local partition order)
        p2idx = np.zeros((len(CHUNKS), 128, P2PAD // 16), np.int16)
        for ci, (ct0, cnt, _, _, _, _) in enumerate(CHUNKS):
            sl = stagidx[ct0 * 128:(ct0 + cnt) * 128].astype(np.int16)
            for j in range((len(sl) + GMAX - 1) // GMAX):
                sub = sl[j * GMAX:(j + 1) * GMAX]
                p2idx[ci, :, j * (GMAX // 16):j * (GMAX // 16) + len(sub) // 16] = \
                    _wrap16(sub, len(sub) // 16)

        wmat = np.zeros((NT, 128, BPT), np.float32)
        wg_t = wg_pad.reshape(NT, BPT, K)
        rows = (np.arange(BPT)[:, None] * K + np.arange(K)[None, :])
        for cb in range(BPT):
            wmat[:, rows[cb], cb] = wg_t[:, cb, :]
        wmat = np.ascontiguousarray(
            wmat.transpose(1, 0, 2).reshape(128, NT * BPT).astype(bf))

        len_c = lengths[bsl]
        m = (np.arange(S)[:, None] < len_c[None, :]).astype(np.float32)
        mask = np.ascontiguousarray(np.broadcast_to(m.reshape(1, NB), (D, NB)))
        mstrip = np.ascontiguousarray(m.T)
        h0T = np.ascontiguousarray(h0[0, bsl].T)

        in_maps.append({
            "table": table_bf,
            "p1idx": p1idx,
            "p2idx": p2idx,
            "wmat": wmat,
            "wihT": wihT,
            "whhT": whhT,
            "bias4": bias4,
            "mask": mask,
            "mstrip": mstrip,
            "h0T": h0T,
        })
    return in_maps


def kernel(items, basket_len, lengths, encode, w_ih, w_hh, b_ih, b_hh, h0,
           _trace=False):
    from concourse.bass_utils import run_bass_kernel_spmd

    nc = _build()
    in_maps = _host_prep(items, basket_len, lengths, encode,
                         w_ih, w_hh, b_ih, b_hh, h0)
    res = run_bass_kernel_spmd(nc, in_maps, core_ids=list(range(NCORES)),
                               trace=_trace)
    y = np.zeros((B, S, D), np.float32)
    h_u = np.zeros((1, B, D), np.float32)
    for c in range(NCORES):
        y[c * BL:(c + 1) * BL] = res.results[c]["y"].reshape(BL, S, D)
        h_u[0, c * BL:(c + 1) * BL] = res.results[c]["hout"]
    if _trace:
        kernel._last_exec_ns = res.exec_time_ns
        kernel._last_res = res
    return y, h_u


# revision 6
# speedup vs baseline: 1.3700x; 1.3700x over previous
"""Trainium2 Bass kernel for DRModel: ragged basket-pool + masked GRU.

Computation (matches the jax reference):
  pooled[b,s,:] = mean over valid k of encode[items[b,s,k]]   (basket pooling)
  GRU over s with packed-sequence masking:
    h' = where(s < len[b], GRUCell(pooled[b,s], h), h)
    y[b,s] = where(s < len[b], h', 0)

Sharding: data-parallel over batch, 32 users per core, 8 cores.
Embedding table (cast to bf16) + GRU weights replicated.

Device strategy per core:
  * Gather: one indirect DMA per 32-tile group (4096 rows) -- amortizes the
    ~1us SWDGE fixed overhead; rows land [token-partition, tile, D].
  * Pooling: PE matmul emb[tokens,D].T-free vs host-built [tokens, 6] block
    weight matrix (basket mask & 1/len folded in) -> psum [D, baskets],
    basket order s-major (col = s*32 + b).
  * gx = W_ih @ x precomputed in 480-col chunks (= 15 GRU steps), biases
    folded; z-gate weights/bias negated so sigmoid gives (1-z) directly.
  * GRU per step: 3 bf16 matmuls (r,z,n) vs h_bf; then
      a_rz = psum_rz + gx_rz ; sig = sigmoid(a_rz)        (DVE, ACT)
      ghn  = psum_n + b_hhn   (ACT Identity+bias, off critical path)
      n    = tanh(sig_r * ghn + gx_n)                     (DVE, DVE, ACT)
      p    = mask * sig_zc                                (GpSimd, off path)
      h'   = h + p * (n - h)                              (DVE x3)
      y_s  = mstrip_s * transpose(h')       (PE transpose + ACT scale-copy)
"""

import numpy as np

B, S, K, D, V = 256, 50, 20, 128, 100000
NCORES = 8
BL = B // NCORES          # 32 users per core
NB = BL * S               # 1600 baskets per core
BPT = 6                   # baskets per token-tile
TPT = BPT * K             # 120 tokens per tile
NT = (NB + BPT - 1) // BPT            # 267 tiles
G = 32                    # tiles per gather group
NG = (NT + G - 1) // G                # 9 groups
NT_PAD = NG * G                       # 288

# chunking: 80 tiles = 480 cols = 15 steps per chunk (last chunk ragged)
CH_TILES = 80
CH_COLS = CH_TILES * BPT              # 480
CH_STEPS = CH_COLS // BL              # 15
CHUNKS = []  # (tile0, ntiles, col0, ncols_pad, step0, nsteps)
t0 = 0
while t0 < NT:
    nt = min(CH_TILES, NT - t0)
    col0 = t0 * BPT
    s0 = col0 // BL
    CHUNKS.append((t0, nt, col0, nt * BPT, s0, min(S - s0, CH_STEPS)))
    t0 += nt

_CACHE = {}


def _build():
    if "nc" in _CACHE:
        return _CACHE["nc"]
    import concourse.bacc as bacc
    import concourse.mybir as mybir
    import concourse.tile as tile
    from concourse import bass
    from concourse.masks import make_identity

    f32 = mybir.dt.float32
    bf16 = mybir.dt.bfloat16
    i32 = mybir.dt.int32
    AF = mybir.ActivationFunctionType

    nc = bacc.Bacc("TRN2", target_bir_lowering=False, debug=False,
                   num_devices=NCORES)

    table = nc.dram_tensor("table", [V, D], bf16, kind="ExternalInput")
    idx_d = nc.dram_tensor("idx", [NG, 128, G], i32, kind="ExternalInput")
    wmat_d = nc.dram_tensor("wmat", [NG, 128, G * BPT], bf16, kind="ExternalInput")
    wihT_d = nc.dram_tensor("wihT", [3, D, D], bf16, kind="ExternalInput")
    whhT_d = nc.dram_tensor("whhT", [3, D, D], bf16, kind="ExternalInput")
    bias_d = nc.dram_tensor("bias4", [D, 4], f32, kind="ExternalInput")
    mask_d = nc.dram_tensor("mask", [D, NB], f32, kind="ExternalInput")
    mstrip_d = nc.dram_tensor("mstrip", [BL, S], f32, kind="ExternalInput")
    h0_d = nc.dram_tensor("h0T", [D, BL], f32, kind="ExternalInput")
    y_d = nc.dram_tensor("y", [BL, S * D], f32, kind="ExternalOutput")
    hout_d = nc.dram_tensor("hout", [BL, D], f32, kind="ExternalOutput")

    with tile.TileContext(nc) as tc:
        with (
            tc.tile_pool(name="const", bufs=1) as cp,
            tc.tile_pool(name="big", bufs=1) as bigp,
            tc.tile_pool(name="emb", bufs=3) as ep,
            tc.tile_pool(name="grp", bufs=2) as gp,
            tc.tile_pool(name="gru", bufs=3) as grp,
            tc.tile_pool(name="hh", bufs=2) as hp,
            tc.tile_pool(name="ppool", bufs=2, space="PSUM") as ppp,
            tc.tile_pool(name="pgx", bufs=2, space="PSUM") as pgx,
            tc.tile_pool(name="pgh", bufs=2, space="PSUM") as pgh,
            tc.tile_pool(name="ptr", bufs=2, space="PSUM") as ptr,
        ):
            # ---- constants ----
            wih_sb = cp.tile([D, 3 * D], bf16, tag="wih")
            whh_sb = cp.tile([D, 3 * D], bf16, tag="whh")
            for g in range(3):
                nc.sync.dma_start(out=wih_sb[:, g * D:(g + 1) * D], in_=wihT_d[g])
                nc.sync.dma_start(out=whh_sb[:, g * D:(g + 1) * D], in_=whhT_d[g])
            bias_sb = cp.tile([D, 4], f32, tag="bias")
            nc.sync.dma_start(out=bias_sb[:], in_=bias_d[:])
            mask_sb = cp.tile([D, NB], f32, tag="mask")
            nc.sync.dma_start(out=mask_sb[:], in_=mask_d[:])
            mstrip_sb = cp.tile([BL, S], f32, tag="mstrip")
            nc.sync.dma_start(out=mstrip_sb[:], in_=mstrip_d[:])
            ident = cp.tile([128, 128], f32, tag="ident")
            make_identity(nc, ident[:])
            h_first = hp.tile([D, BL], f32, tag="h")
            nc.sync.dma_start(out=h_first[:], in_=h0_d[:])

            y_sb = bigp.tile([BL, S * D], f32, tag="y")

            # ---- pooling + gx, chunk by chunk ----
            # gather groups don't align with psum chunks; gather lazily.
            mega = {}   # group -> (tile, ntiles)

            def ensure_group(g):
                if g in mega:
                    return mega[g]
                nrow = min(G, NT - g * G)
                ig = gp.tile([128, G], i32, tag="ig")
                nc.sync.dma_start(out=ig[:], in_=idx_d[g])
                wg = gp.tile([128, G * BPT], bf16, tag="wg")
                nc.sync.dma_start(out=wg[:], in_=wmat_d[g])
                em = ep.tile([128, G * D], bf16, tag="emb")
                for j in range(nrow):
                    nc.gpsimd.indirect_dma_start(
                        out=em[:TPT, j * D:(j + 1) * D],
                        out_offset=None,
                        in_=table[:],
                        in_offset=bass.IndirectOffsetOnAxis(
                            ap=ig[:TPT, j:j + 1], axis=0),
                    )
                mega[g] = (em, wg)
                return mega[g]

            pooled_c = []
            gxrz_c = []
            gxn_c = []
            for (ct0, cnt, col0, ncp, s0, nst) in CHUNKS:
                pp = ppp.tile([D, ncp], f32, tag="pp")
                for j in range(cnt):
                    t = ct0 + j
                    g, jg = divmod(t, G)
                    em, wg = ensure_group(g)
                    nc.tensor.matmul(
                        out=pp[:, j * BPT:(j + 1) * BPT],
                        lhsT=em[:TPT, jg * D:(jg + 1) * D],
                        rhs=wg[:TPT, jg * BPT:(jg + 1) * BPT],
                        start=True, stop=True,
                    )
                    if g * G + G - 1 <= t:
                        mega.pop(g, None)
                pool_sb = bigp.tile([D, ncp], bf16, tag=f"pool{ct0}")
                nc.vector.tensor_copy(pool_sb[:], pp[:])
                pooled_c.append(pool_sb)

                ncols = nst * BL
                gxrz = bigp.tile([D, nst * 2 * BL], f32, tag=f"gxrz{ct0}")
                gxn = bigp.tile([D, ncols], f32, tag=f"gxn{ct0}")
                gxrz_v = gxrz[:].rearrange("p (s h b) -> p s h b", h=2, b=BL)
                for g in range(3):
                    px = pgx.tile([D, ncols], f32, tag="px")
                    nc.tensor.matmul(
                        out=px[:],
                        lhsT=wih_sb[:, g * D:(g + 1) * D],
                        rhs=pool_sb[:, :ncols],
                        start=True, stop=True,
                    )
                    if g < 2:
                        dst = gxrz_v[:, :, g, :]
                    else:
                        dst = gxn[:]
                    nc.vector.tensor_scalar_add(dst, px[:], bias_sb[:, g:g + 1])
                gxrz_c.append(gxrz)
                gxn_c.append(gxn)

            # ---- GRU ----
            h_prev = h_first
            h_bf = hp.tile([D, BL], bf16, tag="hbf")
            nc.vector.tensor_copy(h_bf[:], h_prev[:])
            for s in range(S):
                ci = min(s // CH_STEPS, len(CHUNKS) - 1)
                sl = s - CHUNKS[ci][4]
                gxrz = gxrz_c[ci]
                gxn = gxn_c[ci]

                pg = pgh.tile([D, 3 * BL], f32, tag="pg")
                for g in range(3):
                    nc.tensor.matmul(
                        out=pg[:, g * BL:(g + 1) * BL],
                        lhsT=whh_sb[:, g * D:(g + 1) * D],
                        rhs=h_bf[:],
                        start=True, stop=True,
                    )
                a_rz = grp.tile([D, 2 * BL], f32, tag="a_rz")
                nc.vector.tensor_add(a_rz[:], pg[:, 0:2 * BL],
                                     gxrz[:, sl * 2 * BL:(sl + 1) * 2 * BL])
                sig = grp.tile([D, 2 * BL], f32, tag="sig")
                nc.scalar.activation(sig[:], a_rz[:], AF.Sigmoid)
                ghn = grp.tile([D, BL], f32, tag="ghn")
                nc.scalar.activation(ghn[:], pg[:, 2 * BL:3 * BL], AF.Identity,
                                     bias=bias_sb[:, 3:4])
                p = grp.tile([D, BL], f32, tag="p")
                nc.gpsimd.tensor_mul(p[:], sig[:, BL:2 * BL],
                                     mask_sb[:, s * BL:(s + 1) * BL])
                rn = grp.tile([D, BL], f32, tag="rn")
                nc.vector.tensor_mul(rn[:], sig[:, 0:BL], ghn[:])
                npre = grp.tile([D, BL], f32, tag="npre")
                nc.vector.tensor_add(npre[:], rn[:],
                                     gxn[:, sl * BL:(sl + 1) * BL])
                nt_ = grp.tile([D, BL], f32, tag="nt")
                nc.scalar.activation(nt_[:], npre[:], AF.Tanh)
                u = grp.tile([D, BL], f32, tag="u")
                nc.vector.tensor_sub(u[:], nt_[:], h_prev[:])
                pu = grp.tile([D, BL], f32, tag="pu")
                nc.vector.tensor_mul(pu[:], p[:], u[:])
                h_next = hp.tile([D, BL], f32, tag="h")
                nc.vector.tensor_add(h_next[:], h_prev[:], pu[:])
                h_bf = hp.tile([D, BL], bf16, tag="hbf")
                nc.vector.tensor_copy(h_bf[:], h_next[:])

                tr = ptr.tile([BL, D], f32, tag="tr")
                nc.tensor.transpose(out=tr[:], in_=h_next[:], identity=ident[:])
                nc.scalar.activation(y_sb[:, s * D:(s + 1) * D], tr[:],
                                     AF.Identity, scale=mstrip_sb[:, s:s + 1])
                h_prev = h_next

            hout_sb = cp.tile([BL, D], f32, tag="hout")
            tr = ptr.tile([BL, D], f32, tag="tr")
            nc.tensor.transpose(out=tr[:], in_=h_prev[:], identity=ident[:])
            nc.vector.tensor_copy(hout_sb[:], tr[:])

            nc.sync.dma_start(out=y_d[:], in_=y_sb[:])
            nc.sync.dma_start(out=hout_d[:], in_=hout_sb[:])

    nc.compile()
    _CACHE["nc"] = nc
    return nc


def _host_prep(items, basket_len, lengths, encode, w_ih, w_hh, b_ih, b_hh, h0):
    """Build per-core input maps."""
    import ml_dtypes
    bf = ml_dtypes.bfloat16

    items = np.asarray(items).astype(np.int64)
    basket_len = np.asarray(basket_len).astype(np.int64)
    lengths = np.asarray(lengths).astype(np.int64)
    encode = np.asarray(encode, dtype=np.float32)
    w_ih = np.asarray(w_ih, dtype=np.float32)
    w_hh = np.asarray(w_hh, dtype=np.float32)
    b_ih = np.asarray(b_ih, dtype=np.float32)
    b_hh = np.asarray(b_hh, dtype=np.float32)
    h0 = np.asarray(h0, dtype=np.float32)

    table_bf = np.ascontiguousarray(encode.astype(bf))

    wihT = np.stack([w_ih[g * D:(g + 1) * D].T.copy() for g in range(3)])
    whhT = np.stack([w_hh[g * D:(g + 1) * D].T.copy() for g in range(3)])
    wihT[1] = -wihT[1]
    whhT[1] = -whhT[1]
    bias4 = np.zeros((D, 4), np.float32)
    bias4[:, 0] = b_ih[0:D] + b_hh[0:D]
    bias4[:, 1] = -(b_ih[D:2 * D] + b_hh[D:2 * D])
    bias4[:, 2] = b_ih[2 * D:3 * D]
    bias4[:, 3] = b_hh[2 * D:3 * D]
    wihT = np.ascontiguousarray(wihT.astype(bf))
    whhT = np.ascontiguousarray(whhT.astype(bf))

    karange = np.arange(K)[None, None, :]
    wgt = (karange < basket_len[..., None]).astype(np.float32)
    wgt /= basket_len[..., None].astype(np.float32)

    in_maps = []
    for c in range(NCORES):
        bsl = slice(c * BL, (c + 1) * BL)
        it_c = np.transpose(items[bsl], (1, 0, 2)).reshape(NB, K)
        wg_c = np.transpose(wgt[bsl], (1, 0, 2)).reshape(NB, K)
        it_pad = np.zeros((NT_PAD * BPT, K), np.int64)
        wg_pad = np.zeros((NT_PAD * BPT, K), np.float32)
        it_pad[:NB] = it_c
        wg_pad[:NB] = wg_c
        it_t = it_pad.reshape(NT_PAD, BPT, K)
        wg_t = wg_pad.reshape(NT_PAD, BPT, K)
        idx_rows = np.zeros((NT_PAD, 128), np.int32)
        idx_rows[:, :TPT] = it_t.reshape(NT_PAD, TPT).astype(np.int32)
        idx_g = np.ascontiguousarray(
            idx_rows.reshape(NG, G, 128).transpose(0, 2, 1))
        wmat = np.zeros((NG, G, 128, BPT), np.float32)
        rows = (np.arange(BPT)[:, None] * K + np.arange(K)[None, :])
        for cb in range(BPT):
            wmat[:, :, rows[cb], cb] = wg_t[:, cb, :].reshape(NG, G, K)
        wmat = np.ascontiguousarray(
            wmat.transpose(0, 2, 1, 3).reshape(NG, 128, G * BPT).astype(bf))

        len_c = lengths[bsl]
        m = (np.arange(S)[:, None] < len_c[None, :]).astype(np.float32)
        mask = np.ascontiguousarray(np.broadcast_to(m.reshape(1, NB), (D, NB)))
        mstrip = np.ascontiguousarray(m.T)
        h0T = np.ascontiguousarray(h0[0, bsl].T)

        in_maps.append({
            "table": table_bf,
            "idx": idx_g,
            "wmat": wmat,
            "wihT": wihT,
            "whhT": whhT,
            "bias4": bias4,
            "mask": mask,
            "mstrip": mstrip,
            "h0T": h0T,
        })
    return in_maps


def kernel(items, basket_len, lengths, encode, w_ih, w_hh, b_ih, b_hh, h0,
           _trace=False):
    from concourse.bass_utils import run_bass_kernel_spmd

    nc = _build()
    in_maps = _host_prep(items, basket_len, lengths, encode,
                         w_ih, w_hh, b_ih, b_hh, h0)
    res = run_bass_kernel_spmd(nc, in_maps, core_ids=list(range(NCORES)),
                               trace=_trace)
    y = np.zeros((B, S, D), np.float32)
    h_u = np.zeros((1, B, D), np.float32)
    for c in range(NCORES):
        y[c * BL:(c + 1) * BL] = res.results[c]["y"].reshape(BL, S, D)
        h_u[0, c * BL:(c + 1) * BL] = res.results[c]["hout"]
    if _trace:
        kernel._last_exec_ns = res.exec_time_ns
        kernel._last_res = res
    return y, h_u


# revision 7
# speedup vs baseline: 1.6898x; 1.2334x over previous
"""Trainium2 Bass kernel for DRModel: ragged basket-pool + masked GRU.

Computation (matches the jax reference):
  pooled[b,s,:] = mean over valid k of encode[items[b,s,k]]   (basket pooling)
  GRU over s with packed-sequence masking:
    h' = where(s < len[b], GRUCell(pooled[b,s], h), h)
    y[b,s] = where(s < len[b], h', 0)

Sharding: data-parallel over batch, 32 users per core, 8 cores.
Embedding table (cast to bf16) + GRU weights replicated.

Device strategy per core:
  * Gather: one indirect DMA per 32-tile group (4096 rows) -- amortizes the
    ~1us SWDGE fixed overhead; rows land [token-partition, tile, D].
  * Pooling: PE matmul emb[tokens,D].T-free vs host-built [tokens, 6] block
    weight matrix (basket mask & 1/len folded in) -> psum [D, baskets],
    basket order s-major (col = s*32 + b).
  * gx = W_ih @ x precomputed in 480-col chunks (= 15 GRU steps), biases
    folded; z-gate weights/bias negated so sigmoid gives (1-z) directly.
  * GRU per step: 3 bf16 matmuls (r,z,n) vs h_bf; then
      a_rz = psum_rz + gx_rz ; sig = sigmoid(a_rz)        (DVE, ACT)
      ghn  = psum_n + b_hhn   (ACT Identity+bias, off critical path)
      n    = tanh(sig_r * ghn + gx_n)                     (DVE, DVE, ACT)
      p    = mask * sig_zc                                (GpSimd, off path)
      h'   = h + p * (n - h)                              (DVE x3)
      y_s  = mstrip_s * transpose(h')       (PE transpose + ACT scale-copy)
"""

import numpy as np

B, S, K, D, V = 256, 50, 20, 128, 100000
NCORES = 8
BL = B // NCORES          # 32 users per core
NB = BL * S               # 1600 baskets per core
BPT = 6                   # baskets per token-tile
TPT = BPT * K             # 120 tokens per tile
NT = (NB + BPT - 1) // BPT            # 267 tiles
G = 32                    # tiles per gather group
NG = (NT + G - 1) // G                # 9 groups
NT_PAD = NG * G                       # 288

# chunking: 80 tiles = 480 cols = 15 steps per chunk (last chunk ragged)
CH_TILES = 80
CH_COLS = CH_TILES * BPT              # 480
CH_STEPS = CH_COLS // BL              # 15
CHUNKS = []  # (tile0, ntiles, col0, ncols_pad, step0, nsteps)
t0 = 0
while t0 < NT:
    nt = min(CH_TILES, NT - t0)
    col0 = t0 * BPT
    s0 = col0 // BL
    CHUNKS.append((t0, nt, col0, nt * BPT, s0, min(S - s0, CH_STEPS)))
    t0 += nt

_CACHE = {}


def _build():
    if "nc" in _CACHE:
        return _CACHE["nc"]
    import concourse.bacc as bacc
    import concourse.mybir as mybir
    import concourse.tile as tile
    from concourse import bass
    from concourse.masks import make_identity

    f32 = mybir.dt.float32
    bf16 = mybir.dt.bfloat16
    i32 = mybir.dt.int32
    AF = mybir.ActivationFunctionType

    nc = bacc.Bacc("TRN2", target_bir_lowering=False, debug=False,
                   num_devices=NCORES)

    table = nc.dram_tensor("table", [V, D], bf16, kind="ExternalInput")
    idx_d = nc.dram_tensor("idx", [NG, 128, G], i32, kind="ExternalInput")
    wmat_d = nc.dram_tensor("wmat", [NG, 128, G * BPT], bf16, kind="ExternalInput")
    wihT_d = nc.dram_tensor("wihT", [3, D, D], bf16, kind="ExternalInput")
    whhT_d = nc.dram_tensor("whhT", [3, D, D], bf16, kind="ExternalInput")
    bias_d = nc.dram_tensor("bias4", [D, 4], f32, kind="ExternalInput")
    mask_d = nc.dram_tensor("mask", [D, NB], f32, kind="ExternalInput")
    mstrip_d = nc.dram_tensor("mstrip", [BL, S], f32, kind="ExternalInput")
    h0_d = nc.dram_tensor("h0T", [D, BL], f32, kind="ExternalInput")
    y_d = nc.dram_tensor("y", [BL, S * D], f32, kind="ExternalOutput")
    hout_d = nc.dram_tensor("hout", [BL, D], f32, kind="ExternalOutput")

    with tile.TileContext(nc) as tc:
        with (
            tc.tile_pool(name="const", bufs=1) as cp,
            tc.tile_pool(name="big", bufs=1) as bigp,
            tc.tile_pool(name="emb", bufs=3) as ep,
            tc.tile_pool(name="grp", bufs=2) as gp,
            tc.tile_pool(name="gru", bufs=3) as grp,
            tc.tile_pool(name="hh", bufs=2) as hp,
            tc.tile_pool(name="ppool", bufs=2, space="PSUM") as ppp,
            tc.tile_pool(name="pgx", bufs=2, space="PSUM") as pgx,
            tc.tile_pool(name="pgh", bufs=2, space="PSUM") as pgh,
            tc.tile_pool(name="ptr", bufs=2, space="PSUM") as ptr,
        ):
            # ---- constants ----
            wih_sb = cp.tile([D, 3 * D], bf16, tag="wih")
            whh_sb = cp.tile([D, 3 * D], bf16, tag="whh")
            for g in range(3):
                nc.sync.dma_start(out=wih_sb[:, g * D:(g + 1) * D], in_=wihT_d[g])
                nc.sync.dma_start(out=whh_sb[:, g * D:(g + 1) * D], in_=whhT_d[g])
            bias_sb = cp.tile([D, 4], f32, tag="bias")
            nc.sync.dma_start(out=bias_sb[:], in_=bias_d[:])
            mask_sb = cp.tile([D, NB], f32, tag="mask")
            nc.sync.dma_start(out=mask_sb[:], in_=mask_d[:])
            mstrip_sb = cp.tile([BL, S], f32, tag="mstrip")
            nc.sync.dma_start(out=mstrip_sb[:], in_=mstrip_d[:])
            ident = cp.tile([128, 128], f32, tag="ident")
            make_identity(nc, ident[:])
            h_first = hp.tile([D, BL], f32, tag="h")
            nc.sync.dma_start(out=h_first[:], in_=h0_d[:])

            y_sb = bigp.tile([BL, S * D], f32, tag="y")

            # ---- pooling + gx, chunk by chunk ----
            # gather groups don't align with psum chunks; gather lazily.
            mega = {}   # group -> (tile, ntiles)

            def ensure_group(g):
                if g in mega:
                    return mega[g]
                nrow = min(G, NT - g * G)
                ig = gp.tile([128, G], i32, tag="ig")
                nc.sync.dma_start(out=ig[:], in_=idx_d[g])
                wg = gp.tile([128, G * BPT], bf16, tag="wg")
                nc.sync.dma_start(out=wg[:], in_=wmat_d[g])
                em = ep.tile([128, G * D], bf16, tag="emb")
                for j in range(nrow):
                    nc.gpsimd.indirect_dma_start(
                        out=em[:TPT, j * D:(j + 1) * D],
                        out_offset=None,
                        in_=table[:],
                        in_offset=bass.IndirectOffsetOnAxis(
                            ap=ig[:TPT, j:j + 1], axis=0),
                    )
                mega[g] = (em, wg)
                return mega[g]

            pooled_c = []
            gxrz_c = []
            gxn_c = []
            for (ct0, cnt, col0, ncp, s0, nst) in CHUNKS:
                pp = ppp.tile([D, ncp], f32, tag="pp")
                for j in range(cnt):
                    t = ct0 + j
                    g, jg = divmod(t, G)
                    em, wg = ensure_group(g)
                    nc.tensor.matmul(
                        out=pp[:, j * BPT:(j + 1) * BPT],
                        lhsT=em[:TPT, jg * D:(jg + 1) * D],
                        rhs=wg[:TPT, jg * BPT:(jg + 1) * BPT],
                        start=True, stop=True,
                    )
                    if g * G + G - 1 <= t:
                        mega.pop(g, None)
                pool_sb = bigp.tile([D, ncp], bf16, tag=f"pool{ct0}")
                nc.vector.tensor_copy(pool_sb[:], pp[:])
                pooled_c.append(pool_sb)

                ncols = nst * BL
                gxrz = bigp.tile([D, nst * 2 * BL], f32, tag=f"gxrz{ct0}")
                gxn = bigp.tile([D, ncols], f32, tag=f"gxn{ct0}")
                gxrz_v = gxrz[:].rearrange("p (s h b) -> p s h b", h=2, b=BL)
                for g in range(3):
                    px = pgx.tile([D, ncols], f32, tag="px")
                    nc.tensor.matmul(
                        out=px[:],
                        lhsT=wih_sb[:, g * D:(g + 1) * D],
                        rhs=pool_sb[:, :ncols],
                        start=True, stop=True,
                    )
                    if g < 2:
                        dst = gxrz_v[:, :, g, :]
                    else:
                        dst = gxn[:]
                    nc.vector.tensor_scalar_add(dst, px[:], bias_sb[:, g:g + 1])
                gxrz_c.append(gxrz)
                gxn_c.append(gxn)

            # ---- GRU ----
            h_prev = h_first
            h_bf = hp.tile([D, BL], bf16, tag="hbf")
            nc.vector.tensor_copy(h_bf[:], h_prev[:])
            for s in range(S):
                ci = min(s // CH_STEPS, len(CHUNKS) - 1)
                sl = s - CHUNKS[ci][4]
                gxrz = gxrz_c[ci]
                gxn = gxn_c[ci]

                pg = pgh.tile([D, 3 * BL], f32, tag="pg")
                for g in range(3):
                    nc.tensor.matmul(
                        out=pg[:, g * BL:(g + 1) * BL],
                        lhsT=whh_sb[:, g * D:(g + 1) * D],
                        rhs=h_bf[:],
                        start=True, stop=True,
                    )
                a_rz = grp.tile([D, 2 * BL], f32, tag="a_rz")
                nc.vector.tensor_add(a_rz[:], pg[:, 0:2 * BL],
                                     gxrz[:, sl * 2 * BL:(sl + 1) * 2 * BL])
                sig = grp.tile([D, 2 * BL], f32, tag="sig")
                nc.scalar.activation(sig[:], a_rz[:], AF.Sigmoid)
                ghn = grp.tile([D, BL], f32, tag="ghn")
                nc.scalar.activation(ghn[:], pg[:, 2 * BL:3 * BL], AF.Identity,
                                     bias=bias_sb[:, 3:4])
                rn = grp.tile([D, BL], f32, tag="rn")
                nc.vector.tensor_mul(rn[:], sig[:, 0:BL], ghn[:])
                npre = grp.tile([D, BL], f32, tag="npre")
                nc.vector.tensor_add(npre[:], rn[:],
                                     gxn[:, sl * BL:(sl + 1) * BL])
                p = grp.tile([D, BL], f32, tag="p")
                nc.vector.tensor_mul(p[:], sig[:, BL:2 * BL],
                                     mask_sb[:, s * BL:(s + 1) * BL])
                nt_ = grp.tile([D, BL], f32, tag="nt")
                nc.scalar.activation(nt_[:], npre[:], AF.Tanh)
                u = grp.tile([D, BL], f32, tag="u")
                nc.vector.tensor_sub(u[:], nt_[:], h_prev[:])
                pu = grp.tile([D, BL], f32, tag="pu")
                nc.vector.tensor_mul(pu[:], p[:], u[:])
                h_next = hp.tile([D, BL], f32, tag="h")
                nc.vector.tensor_add(h_next[:], h_prev[:], pu[:])
                h_bf = hp.tile([D, BL], bf16, tag="hbf")
                nc.vector.tensor_copy(h_bf[:], h_next[:])

                tr = ptr.tile([BL, D], f32, tag="tr")
                nc.tensor.transpose(out=tr[:], in_=h_next[:], identity=ident[:])
                nc.scalar.activation(y_sb[:, s * D:(s + 1) * D], tr[:],
                                     AF.Identity, scale=mstrip_sb[:, s:s + 1])
                h_prev = h_next

            hout_sb = cp.tile([BL, D], f32, tag="hout")
            tr = ptr.tile([BL, D], f32, tag="tr")
            nc.tensor.transpose(out=tr[:], in_=h_prev[:], identity=ident[:])
            nc.vector.tensor_copy(hout_sb[:], tr[:])

            nc.sync.dma_start(out=y_d[:], in_=y_sb[:])
            nc.sync.dma_start(out=hout_d[:], in_=hout_sb[:])

    nc.compile()
    _CACHE["nc"] = nc
    return nc


def _host_prep(items, basket_len, lengths, encode, w_ih, w_hh, b_ih, b_hh, h0):
    """Build per-core input maps."""
    import ml_dtypes
    bf = ml_dtypes.bfloat16

    items = np.asarray(items).astype(np.int64)
    basket_len = np.asarray(basket_len).astype(np.int64)
    lengths = np.asarray(lengths).astype(np.int64)
    encode = np.asarray(encode, dtype=np.float32)
    w_ih = np.asarray(w_ih, dtype=np.float32)
    w_hh = np.asarray(w_hh, dtype=np.float32)
    b_ih = np.asarray(b_ih, dtype=np.float32)
    b_hh = np.asarray(b_hh, dtype=np.float32)
    h0 = np.asarray(h0, dtype=np.float32)

    table_bf = np.ascontiguousarray(encode.astype(bf))

    wihT = np.stack([w_ih[g * D:(g + 1) * D].T.copy() for g in range(3)])
    whhT = np.stack([w_hh[g * D:(g + 1) * D].T.copy() for g in range(3)])
    wihT[1] = -wihT[1]
    whhT[1] = -whhT[1]
    bias4 = np.zeros((D, 4), np.float32)
    bias4[:, 0] = b_ih[0:D] + b_hh[0:D]
    bias4[:, 1] = -(b_ih[D:2 * D] + b_hh[D:2 * D])
    bias4[:, 2] = b_ih[2 * D:3 * D]
    bias4[:, 3] = b_hh[2 * D:3 * D]
    wihT = np.ascontiguousarray(wihT.astype(bf))
    whhT = np.ascontiguousarray(whhT.astype(bf))

    karange = np.arange(K)[None, None, :]
    wgt = (karange < basket_len[..., None]).astype(np.float32)
    wgt /= basket_len[..., None].astype(np.float32)

    in_maps = []
    for c in range(NCORES):
        bsl = slice(c * BL, (c + 1) * BL)
        it_c = np.transpose(items[bsl], (1, 0, 2)).reshape(NB, K)
        wg_c = np.transpose(wgt[bsl], (1, 0, 2)).reshape(NB, K)
        it_pad = np.zeros((NT_PAD * BPT, K), np.int64)
        wg_pad = np.zeros((NT_PAD * BPT, K), np.float32)
        it_pad[:NB] = it_c
        wg_pad[:NB] = wg_c
        it_t = it_pad.reshape(NT_PAD, BPT, K)
        wg_t = wg_pad.reshape(NT_PAD, BPT, K)
        idx_rows = np.zeros((NT_PAD, 128), np.int32)
        idx_rows[:, :TPT] = it_t.reshape(NT_PAD, TPT).astype(np.int32)
        idx_g = np.ascontiguousarray(
            idx_rows.reshape(NG, G, 128).transpose(0, 2, 1))
        wmat = np.zeros((NG, G, 128, BPT), np.float32)
        rows = (np.arange(BPT)[:, None] * K + np.arange(K)[None, :])
        for cb in range(BPT):
            wmat[:, :, rows[cb], cb] = wg_t[:, cb, :].reshape(NG, G, K)
        wmat = np.ascontiguousarray(
            wmat.transpose(0, 2, 1, 3).reshape(NG, 128, G * BPT).astype(bf))

        len_c = lengths[bsl]
        m = (np.arange(S)[:, None] < len_c[None, :]).astype(np.float32)
        mask = np.ascontiguousarray(np.broadcast_to(m.reshape(1, NB), (D, NB)))
        mstrip = np.ascontiguousarray(m.T)
        h0T = np.ascontiguousarray(h0[0, bsl].T)

        in_maps.append({
            "table": table_bf,
            "idx": idx_g,
            "wmat": wmat,
            "wihT": wihT,
            "whhT": whhT,
            "bias4": bias4,
            "mask": mask,
            "mstrip": mstrip,
            "h0T": h0T,
        })
    return in_maps


def kernel(items, basket_len, lengths, encode, w_ih, w_hh, b_ih, b_hh, h0,
           _trace=False):
    from concourse.bass_utils import run_bass_kernel_spmd

    nc = _build()
    in_maps = _host_prep(items, basket_len, lengths, encode,
                         w_ih, w_hh, b_ih, b_hh, h0)
    res = run_bass_kernel_spmd(nc, in_maps, core_ids=list(range(NCORES)),
                               trace=_trace)
    y = np.zeros((B, S, D), np.float32)
    h_u = np.zeros((1, B, D), np.float32)
    for c in range(NCORES):
        y[c * BL:(c + 1) * BL] = res.results[c]["y"].reshape(BL, S, D)
        h_u[0, c * BL:(c + 1) * BL] = res.results[c]["hout"]
    if _trace:
        kernel._last_exec_ns = res.exec_time_ns
        kernel._last_res = res
    return y, h_u


# revision 9
# speedup vs baseline: 1.6966x; 1.0040x over previous
"""Trainium2 Bass kernel for DRModel: ragged basket-pool + masked GRU.

Computation (matches the jax reference):
  pooled[b,s,:] = mean over valid k of encode[items[b,s,k]]   (basket pooling)
  GRU over s with packed-sequence masking:
    h' = where(s < len[b], GRUCell(pooled[b,s], h), h)
    y[b,s] = where(s < len[b], h', 0)

Sharding: data-parallel over batch, 32 users per core, 8 cores.
Embedding table (cast to bf16) + GRU weights replicated.

Device strategy per core:
  * Gather: one indirect DMA per 32-tile group (4096 rows) -- amortizes the
    ~1us SWDGE fixed overhead; rows land [token-partition, tile, D].
  * Pooling: PE matmul emb[tokens,D].T-free vs host-built [tokens, 6] block
    weight matrix (basket mask & 1/len folded in) -> psum [D, baskets],
    basket order s-major (col = s*32 + b).
  * gx = W_ih @ x precomputed in 480-col chunks (= 15 GRU steps), biases
    folded; z-gate weights/bias negated so sigmoid gives (1-z) directly.
  * GRU per step: 3 bf16 matmuls (r,z,n) vs h_bf; then
      a_rz = psum_rz + gx_rz ; sig = sigmoid(a_rz)        (DVE, ACT)
      ghn  = psum_n + b_hhn   (ACT Identity+bias, off critical path)
      n    = tanh(sig_r * ghn + gx_n)                     (DVE, DVE, ACT)
      p    = mask * sig_zc                                (GpSimd, off path)
      h'   = h + p * (n - h)                              (DVE x3)
      y_s  = mstrip_s * transpose(h')       (PE transpose + ACT scale-copy)
"""

import numpy as np

B, S, K, D, V = 256, 50, 20, 128, 100000
NCORES = 8
BL = B // NCORES          # 32 users per core
NB = BL * S               # 1600 baskets per core
BPT = 6                   # baskets per token-tile
TPT = BPT * K             # 120 tokens per tile
NT = (NB + BPT - 1) // BPT            # 267 tiles
G = 32                    # tiles per gather group
NG = (NT + G - 1) // G                # 9 groups
NT_PAD = NG * G                       # 288

# chunking: 80 tiles = 480 cols = 15 steps per chunk (last chunk ragged)
CH_TILES = 80
CH_COLS = CH_TILES * BPT              # 480
CH_STEPS = CH_COLS // BL              # 15
CHUNKS = []  # (tile0, ntiles, col0, ncols_pad, step0, nsteps)
t0 = 0
while t0 < NT:
    nt = min(CH_TILES, NT - t0)
    col0 = t0 * BPT
    s0 = col0 // BL
    CHUNKS.append((t0, nt, col0, nt * BPT, s0, min(S - s0, CH_STEPS)))
    t0 += nt

_CACHE = {}


def _build():
    if "nc" in _CACHE:
        return _CACHE["nc"]
    import concourse.bacc as bacc
    import concourse.mybir as mybir
    import concourse.tile as tile
    from concourse import bass
    from concourse.masks import make_identity

    f32 = mybir.dt.float32
    bf16 = mybir.dt.bfloat16
    i32 = mybir.dt.int32
    AF = mybir.ActivationFunctionType

    nc = bacc.Bacc("TRN2", target_bir_lowering=False, debug=False,
                   num_devices=NCORES)

    table = nc.dram_tensor("table", [V, D], bf16, kind="ExternalInput")
    idx_d = nc.dram_tensor("idx", [NG, 128, G], i32, kind="ExternalInput")
    wmat_d = nc.dram_tensor("wmat", [NG, 128, G * BPT], bf16, kind="ExternalInput")
    wihT_d = nc.dram_tensor("wihT", [3, D, D], bf16, kind="ExternalInput")
    whhT_d = nc.dram_tensor("whhT", [3, D, D], bf16, kind="ExternalInput")
    bias_d = nc.dram_tensor("bias4", [D, 4], f32, kind="ExternalInput")
    mask_d = nc.dram_tensor("mask", [D, NB], f32, kind="ExternalInput")
    mstrip_d = nc.dram_tensor("mstrip", [BL, S], f32, kind="ExternalInput")
    h0_d = nc.dram_tensor("h0T", [D, BL], f32, kind="ExternalInput")
    y_d = nc.dram_tensor("y", [BL, S * D], f32, kind="ExternalOutput")
    hout_d = nc.dram_tensor("hout", [BL, D], f32, kind="ExternalOutput")

    with tile.TileContext(nc) as tc:
        with (
            tc.tile_pool(name="const", bufs=1) as cp,
            tc.tile_pool(name="big", bufs=1) as bigp,
            tc.tile_pool(name="emb", bufs=3) as ep,
            tc.tile_pool(name="grp", bufs=2) as gp,
            tc.tile_pool(name="gru", bufs=3) as grp,
            tc.tile_pool(name="hh", bufs=2) as hp,
            tc.tile_pool(name="ppool", bufs=2, space="PSUM") as ppp,
            tc.tile_pool(name="pgx", bufs=2, space="PSUM") as pgx,
            tc.tile_pool(name="pgh", bufs=2, space="PSUM") as pgh,
            tc.tile_pool(name="ptr", bufs=2, space="PSUM") as ptr,
        ):
            # ---- constants ----
            wih_sb = cp.tile([D, 3 * D], bf16, tag="wih")
            whh_sb = cp.tile([D, 3 * D], bf16, tag="whh")
            for g in range(3):
                nc.sync.dma_start(out=wih_sb[:, g * D:(g + 1) * D], in_=wihT_d[g])
                nc.sync.dma_start(out=whh_sb[:, g * D:(g + 1) * D], in_=whhT_d[g])
            bias_sb = cp.tile([D, 4], f32, tag="bias")
            nc.sync.dma_start(out=bias_sb[:], in_=bias_d[:])
            mask_sb = cp.tile([D, NB], f32, tag="mask")
            nc.sync.dma_start(out=mask_sb[:], in_=mask_d[:])
            mstrip_sb = cp.tile([BL, S], f32, tag="mstrip")
            nc.sync.dma_start(out=mstrip_sb[:], in_=mstrip_d[:])
            ident = cp.tile([128, 128], f32, tag="ident")
            make_identity(nc, ident[:])
            h_first = hp.tile([D, BL], f32, tag="h")
            nc.sync.dma_start(out=h_first[:], in_=h0_d[:])

            y_sb = bigp.tile([BL, S * D], f32, tag="y")

            # ---- pooling + gx, chunk by chunk ----
            # gather groups don't align with psum chunks; gather lazily.
            mega = {}   # group -> (tile, ntiles)

            def ensure_group(g):
                if g in mega:
                    return mega[g]
                nrow = min(G, NT - g * G)
                ig = gp.tile([128, G], i32, tag="ig")
                nc.sync.dma_start(out=ig[:], in_=idx_d[g])
                wg = gp.tile([128, G * BPT], bf16, tag="wg")
                nc.sync.dma_start(out=wg[:], in_=wmat_d[g])
                em = ep.tile([128, G * D], bf16, tag="emb")
                for j in range(nrow):
                    nc.gpsimd.indirect_dma_start(
                        out=em[:TPT, j * D:(j + 1) * D],
                        out_offset=None,
                        in_=table[:],
                        in_offset=bass.IndirectOffsetOnAxis(
                            ap=ig[:TPT, j:j + 1], axis=0),
                    )
                mega[g] = (em, wg)
                return mega[g]

            h_prev = h_first
            h_bf = hp.tile([D, BL], mybir.dt.bfloat16, tag="hbf")
            nc.vector.tensor_copy(h_bf[:], h_first[:])

            # ---- GRU: emitted per chunk so it overlaps later pooling ----
            def gru_steps(ci):
                nonlocal h_prev, h_bf
                (ct0, cnt, col0, ncp, s0, nst) = CHUNKS[ci]
                gxrz = gxrz_c[ci]
                gxn = gxn_c[ci]
                for sl in range(nst):
                    s = s0 + sl
                    pg = pgh.tile([D, 3 * BL], f32, tag="pg")
                    for g in range(3):
                        nc.tensor.matmul(
                            out=pg[:, g * BL:(g + 1) * BL],
                            lhsT=whh_sb[:, g * D:(g + 1) * D],
                            rhs=h_bf[:],
                            start=True, stop=True,
                        )
                    a_rz = grp.tile([D, 2 * BL], f32, tag="a_rz")
                    nc.vector.tensor_add(a_rz[:], pg[:, 0:2 * BL],
                                         gxrz[:, sl * 2 * BL:(sl + 1) * 2 * BL])
                    sig = grp.tile([D, 2 * BL], f32, tag="sig")
                    nc.scalar.activation(sig[:], a_rz[:], AF.Sigmoid)
                    ghn = grp.tile([D, BL], f32, tag="ghn")
                    nc.scalar.activation(ghn[:], pg[:, 2 * BL:3 * BL],
                                         AF.Identity, bias=bias_sb[:, 3:4])
                    rn = grp.tile([D, BL], f32, tag="rn")
                    nc.vector.tensor_mul(rn[:], sig[:, 0:BL], ghn[:])
                    npre = grp.tile([D, BL], f32, tag="npre")
                    nc.vector.tensor_add(npre[:], rn[:],
                                         gxn[:, sl * BL:(sl + 1) * BL])
                    p = grp.tile([D, BL], f32, tag="p")
                    nc.vector.tensor_mul(p[:], sig[:, BL:2 * BL],
                                         mask_sb[:, s * BL:(s + 1) * BL])
                    nt_ = grp.tile([D, BL], f32, tag="nt")
                    nc.scalar.activation(nt_[:], npre[:], AF.Tanh)
                    u = grp.tile([D, BL], f32, tag="u")
                    nc.vector.tensor_sub(u[:], nt_[:], h_prev[:])
                    pu = grp.tile([D, BL], f32, tag="pu")
                    nc.vector.tensor_mul(pu[:], p[:], u[:])
                    h_next = hp.tile([D, BL], f32, tag="h")
                    nc.vector.tensor_add(h_next[:], h_prev[:], pu[:])
                    h_bf = hp.tile([D, BL], mybir.dt.bfloat16, tag="hbf")
                    nc.vector.tensor_copy(h_bf[:], h_next[:])

                    tr = ptr.tile([BL, D], f32, tag="tr")
                    nc.tensor.transpose(out=tr[:], in_=h_next[:],
                                        identity=ident[:])
                    nc.scalar.activation(y_sb[:, s * D:(s + 1) * D], tr[:],
                                         AF.Identity, scale=mstrip_sb[:, s:s + 1])
                    h_prev = h_next

            pooled_c = []
            gxrz_c = []
            gxn_c = []
            for ci, (ct0, cnt, col0, ncp, s0, nst) in enumerate(CHUNKS):
                pp = ppp.tile([D, ncp], f32, tag="pp")
                for j in range(cnt):
                    t = ct0 + j
                    g, jg = divmod(t, G)
                    em, wg = ensure_group(g)
                    nc.tensor.matmul(
                        out=pp[:, j * BPT:(j + 1) * BPT],
                        lhsT=em[:TPT, jg * D:(jg + 1) * D],
                        rhs=wg[:TPT, jg * BPT:(jg + 1) * BPT],
                        start=True, stop=True,
                    )
                    if g * G + G - 1 <= t:
                        mega.pop(g, None)
                pool_sb = bigp.tile([D, ncp], bf16, tag=f"pool{ct0}")
                nc.vector.tensor_copy(pool_sb[:], pp[:])
                pooled_c.append(pool_sb)

                ncols = nst * BL
                gxrz = bigp.tile([D, nst * 2 * BL], f32, tag=f"gxrz{ct0}")
                gxn = bigp.tile([D, ncols], f32, tag=f"gxn{ct0}")
                gxrz_v = gxrz[:].rearrange("p (s h b) -> p s h b", h=2, b=BL)
                for g in range(3):
                    px = pgx.tile([D, ncols], f32, tag="px")
                    nc.tensor.matmul(
                        out=px[:],
                        lhsT=wih_sb[:, g * D:(g + 1) * D],
                        rhs=pool_sb[:, :ncols],
                        start=True, stop=True,
                    )
                    if g < 2:
                        dst = gxrz_v[:, :, g, :]
                    else:
                        dst = gxn[:]
                    nc.vector.tensor_scalar_add(dst, px[:], bias_sb[:, g:g + 1])
                gxrz_c.append(gxrz)
                gxn_c.append(gxn)
                gru_steps(ci)

            hout_sb = cp.tile([BL, D], f32, tag="hout")
            tr = ptr.tile([BL, D], f32, tag="tr")
            nc.tensor.transpose(out=tr[:], in_=h_prev[:], identity=ident[:])
            nc.vector.tensor_copy(hout_sb[:], tr[:])

            nc.sync.dma_start(out=y_d[:], in_=y_sb[:])
            nc.sync.dma_start(out=hout_d[:], in_=hout_sb[:])

    nc.compile()
    _CACHE["nc"] = nc
    return nc


def _host_prep(items, basket_len, lengths, encode, w_ih, w_hh, b_ih, b_hh, h0):
    """Build per-core input maps."""
    import ml_dtypes
    bf = ml_dtypes.bfloat16

    items = np.asarray(items).astype(np.int64)
    basket_len = np.asarray(basket_len).astype(np.int64)
    lengths = np.asarray(lengths).astype(np.int64)
    encode = np.asarray(encode, dtype=np.float32)
    w_ih = np.asarray(w_ih, dtype=np.float32)
    w_hh = np.asarray(w_hh, dtype=np.float32)
    b_ih = np.asarray(b_ih, dtype=np.float32)
    b_hh = np.asarray(b_hh, dtype=np.float32)
    h0 = np.asarray(h0, dtype=np.float32)

    table_bf = np.ascontiguousarray(encode.astype(bf))

    wihT = np.stack([w_ih[g * D:(g + 1) * D].T.copy() for g in range(3)])
    whhT = np.stack([w_hh[g * D:(g + 1) * D].T.copy() for g in range(3)])
    wihT[1] = -wihT[1]
    whhT[1] = -whhT[1]
    bias4 = np.zeros((D, 4), np.float32)
    bias4[:, 0] = b_ih[0:D] + b_hh[0:D]
    bias4[:, 1] = -(b_ih[D:2 * D] + b_hh[D:2 * D])
    bias4[:, 2] = b_ih[2 * D:3 * D]
    bias4[:, 3] = b_hh[2 * D:3 * D]
    wihT = np.ascontiguousarray(wihT.astype(bf))
    whhT = np.ascontiguousarray(whhT.astype(bf))

    karange = np.arange(K)[None, None, :]
    wgt = (karange < basket_len[..., None]).astype(np.float32)
    wgt /= basket_len[..., None].astype(np.float32)

    in_maps = []
    for c in range(NCORES):
        bsl = slice(c * BL, (c + 1) * BL)
        it_c = np.transpose(items[bsl], (1, 0, 2)).reshape(NB, K)
        wg_c = np.transpose(wgt[bsl], (1, 0, 2)).reshape(NB, K)
        it_pad = np.zeros((NT_PAD * BPT, K), np.int64)
        wg_pad = np.zeros((NT_PAD * BPT, K), np.float32)
        it_pad[:NB] = it_c
        wg_pad[:NB] = wg_c
        it_t = it_pad.reshape(NT_PAD, BPT, K)
        wg_t = wg_pad.reshape(NT_PAD, BPT, K)
        idx_rows = np.zeros((NT_PAD, 128), np.int32)
        idx_rows[:, :TPT] = it_t.reshape(NT_PAD, TPT).astype(np.int32)
        idx_g = np.ascontiguousarray(
            idx_rows.reshape(NG, G, 128).transpose(0, 2, 1))
        wmat = np.zeros((NG, G, 128, BPT), np.float32)
        rows = (np.arange(BPT)[:, None] * K + np.arange(K)[None, :])
        for cb in range(BPT):
            wmat[:, :, rows[cb], cb] = wg_t[:, cb, :].reshape(NG, G, K)
        wmat = np.ascontiguousarray(
            wmat.transpose(0, 2, 1, 3).reshape(NG, 128, G * BPT).astype(bf))

        len_c = lengths[bsl]
        m = (np.arange(S)[:, None] < len_c[None, :]).astype(np.float32)
        mask = np.ascontiguousarray(np.broadcast_to(m.reshape(1, NB), (D, NB)))
        mstrip = np.ascontiguousarray(m.T)
        h0T = np.ascontiguousarray(h0[0, bsl].T)

        in_maps.append({
            "table": table_bf,
            "idx": idx_g,
            "wmat": wmat,
            "wihT": wihT,
            "whhT": whhT,
            "bias4": bias4,
            "mask": mask,
            "mstrip": mstrip,
            "h0T": h0T,
        })
    return in_maps


def kernel(items, basket_len, lengths, encode, w_ih, w_hh, b_ih, b_hh, h0,
           _trace=False):
    from concourse.bass_utils import run_bass_kernel_spmd

    nc = _build()
    in_maps = _host_prep(items, basket_len, lengths, encode,
                         w_ih, w_hh, b_ih, b_hh, h0)
    res = run_bass_kernel_spmd(nc, in_maps, core_ids=list(range(NCORES)),
                               trace=_trace)
    y = np.zeros((B, S, D), np.float32)
    h_u = np.zeros((1, B, D), np.float32)
    for c in range(NCORES):
        y[c * BL:(c + 1) * BL] = res.results[c]["y"].reshape(BL, S, D)
        h_u[0, c * BL:(c + 1) * BL] = res.results[c]["hout"]
    if _trace:
        kernel._last_exec_ns = res.exec_time_ns
        kernel._last_res = res
    return y, h_u
